# revision 37
# baseline (speedup 1.0000x reference)
"""Trainium2 Bass kernel for 3D volume attention (b=2, x=y=z=16, c=64,
heads=4, dim_head=32, qk-standardize over sequence, scale=16).

Sharding: batch*heads = 8 (b,h) pairs -> 8 NeuronCores, one pair per core.
Host pre-transposes x and pre-slices per-head weights; host sums the 4
head-partials per batch (pure unshard-reduce) and reshapes.

v2 design (vs v1 461us baseline):
  - standardize folded into projection weights: stats come from
    C_aug = sum_s [x;1][x]^T computed on the PE, so mu/var/rstd exist
    BEFORE the q/k projections and (q-mu)*rstd*16 collapses into a
    scaled weight + bias row. Kills the serial proj->stats->replica
    prologue chain.
  - fp16 single-product q.k (abs logit err ~0.1, sim rel-err 3.2e-3)
    replaces bf16 hi/lo: pass-B K drops 97->33, allowing 2-way
    row-tile packing at tile_position (0,0)/(64,0) -> 2x pass-B wall.
  - AV pair column-packed at (0,0)/(0,64) into ONE psum bank
    (partitions 0-32 / 64-96) -> 2x AV wall, frees a psum bank.
  - max-scan split: most i-blocks DVE reduce_max, a few on ACT
    exp(s/16)-accum lse (keeps both engines ~equally loaded).
  - out-stage (1/l, normalize, w_out projection, DMA) pipelined per
    chunk, borrowing psB psum slots at chunk boundaries.
"""
import os
import sys
from contextlib import ExitStack

import numpy as np

_PROBLEM_DIR = os.path.dirname(os.path.abspath(__file__))
if _PROBLEM_DIR not in sys.path:
    sys.path.insert(0, _PROBLEM_DIR)

import concourse.bass as bass
import concourse.tile as tile
from concourse import bacc, mybir
from concourse.bass_utils import run_bass_kernel_spmd

F32 = mybir.dt.float32
F32R = mybir.dt.float32r
BF16 = mybir.dt.bfloat16
F16 = mybir.dt.float16
AF = mybir.ActivationFunctionType
ALU = mybir.AluOpType

HEADS = 4
DH = 32
CIN = 64
S = 4096
SCALE = 16.0
EPS = 1e-5
NB = S // 128     # 32 j blocks
NCH = 4
CHUNK = 1024
NLSE = int(os.environ.get("NLSE", "4"))
STAGE = int(os.environ.get("STAGE", "4"))  # i-blocks (of 32) scanned via ACT-lse

_compiled = None


def _build():
    nc = bacc.Bacc("TRN2", target_bir_lowering=False, debug=False, num_devices=8)
    xTa_d = nc.dram_tensor("xTa", [CIN + 1, S], F32, kind="ExternalInput").ap()
    xNb_d = nc.dram_tensor("xNb", [128, 65 * 32], F32, kind="ExternalInput").ap()
    wqT_d = nc.dram_tensor("wqT", [128, CIN], F32, kind="ExternalInput").ap()
    wkT_d = nc.dram_tensor("wkT", [128, CIN], F32, kind="ExternalInput").ap()
    wv_d = nc.dram_tensor("wv", [CIN, DH], F32, kind="ExternalInput").ap()
    wo_d = nc.dram_tensor("wo", [DH + 1, CIN], F32, kind="ExternalInput").ap()
    out_d = nc.dram_tensor("out", [CIN, S], F32, kind="ExternalOutput").ap()

    with tile.TileContext(nc) as tc, ExitStack() as ctx:
        per = ctx.enter_context(tc.tile_pool(name="per", bufs=1))

        # ---- persistent SBUF ----
        wo_r = per.tile([DH + 1, CIN], F32R)
        qF = per.tile([128, S], F16)       # 4 bands of qhat*16 (pass A stat)
        kF = per.tile([128, S], F16)       # 4 bands of khat    (pass A mov)
        kP = per.tile([97, S], F16)        # rows 0-31 khat, 32=-1; replica @64
        qPc = [per.tile([97, CHUNK], F16, name=f"qPc{c}") for c in range(NCH)]
        vaug = per.tile([128, 33 * NB], BF16)
        ident = per.tile([128, 128], F32)
        ones33_f = per.tile([1, 33], F32)
        ones33 = per.tile([1, 33], F32R)
        m8 = [per.tile([128, 8], F32, name=f"m8_{c}") for c in range(NCH)]
        lse_bias = per.tile([128, 1], F32)
        yT = per.tile([CIN, CHUNK], F32)

        from concourse.masks import make_identity

        # ================= prologue =================
        with tc.tile_pool(name="prow", bufs=1) as prow, \
             tc.tile_pool(name="props", bufs=1, space="PSUM") as props:
            xTa = prow.tile([CIN + 1, S], F32)
            xNb = prow.tile([128, 65 * 32], F32)
            nc.sync.dma_start(xNb[:], xNb_d[:])
            nc.sync.dma_start(xTa[:], xTa_d[:])
            wqT = prow.tile([128, CIN], F32)
            wkT = prow.tile([128, CIN], F32)
            wv = prow.tile([CIN, DH], F32)
            wo = prow.tile([DH + 1, CIN], F32)
            nc.sync.dma_start(wqT[:], wqT_d[:])
            nc.sync.dma_start(wkT[:], wkT_d[:])
            nc.sync.dma_start(wv[:], wv_d[:])
            nc.sync.dma_start(wo[:], wo_d[:])
            nc.vector.tensor_copy(wo_r[:], wo[:])
            make_identity(nc, ident[:])
            nc.vector.memset(ones33_f[:], 1.0)
            nc.vector.tensor_copy(ones33[:], ones33_f[:])
            nc.vector.memset(vaug[:], 1.0)
            nc.vector.memset(lse_bias[:], -31.25)

            xTr = prow.tile([CIN + 1, S], F32R)
            nc.vector.tensor_copy(xTr[:], xTa[:])

            # ---- C_aug = sum_s [x][x;1]^T : [64, 65] ----
            pC = props.tile([CIN, 512], F32, name="pC", tag="st", bufs=2)
            for sb in range(32):
                nc.tensor.matmul(pC[:, 0:65], xNb[:, bass.ds(65 * sb, CIN)],
                                 xNb[:, bass.ds(65 * sb, 65)],
                                 start=(sb == 0), stop=(sb == 31))
            C_sb = prow.tile([CIN, 65], F32R)
            nc.scalar.copy(C_sb[:], pC[:, 0:65])

            # ---- wq/wk natural layout [64, 128] via PE transpose ----
            pWq = props.tile([CIN, 512], F32, name="pWq", tag="st", bufs=2)
            wq_sb = prow.tile([CIN, 128], F32R)
            wk_sb = prow.tile([CIN, 128], F32R)
            nc.tensor.transpose(pWq[:, 0:128], wqT[:], ident[:])
            nc.scalar.copy(wq_sb[:], pWq[:, 0:128])
            pWk = props.tile([CIN, 512], F32, name="pWk", tag="st", bufs=2)
            nc.tensor.transpose(pWk[:, 0:128], wkT[:], ident[:])
            nc.scalar.copy(wk_sb[:], pWk[:, 0:128])

            # ---- T_aug = C_aug^T @ w : rows 0-63 = C w, row 64 = S*mu ----
            stc = prow.tile([128, 4], F32)   # cols: e2q, e2k, Smu_q, Smu_k
            wT_q = prow.tile([CIN, 128], F32R)
            wT_k = prow.tile([CIN, 128], F32R)
            ones64_f = prow.tile([CIN, 1], F32)
            ones64 = prow.tile([CIN, 1], F32R)
            nc.vector.memset(ones64_f[:], 1.0)
            nc.vector.tensor_copy(ones64[:], ones64_f[:])
            srow = [prow.tile([1, 128], F32, name=f"srow{i}") for i in range(4)]
            pTq = props.tile([65, 512], F32, name="pTq", tag="st", bufs=2)
            nc.tensor.matmul(pTq[:, 0:128], C_sb[:], wq_sb[:], start=True, stop=True)
            nc.vector.tensor_tensor(out=wT_q[:], in0=wq_sb[:], in1=pTq[0:CIN, 0:128],
                                    op=ALU.mult)
            nc.scalar.copy(srow[2][:], pTq[CIN:CIN + 1, 0:128])
            pTk = props.tile([65, 512], F32, name="pTk", tag="st", bufs=2)
            nc.tensor.matmul(pTk[:, 0:128], C_sb[:], wk_sb[:], start=True, stop=True)
            nc.vector.tensor_tensor(out=wT_k[:], in0=wk_sb[:], in1=pTk[0:CIN, 0:128],
                                    op=ALU.mult)
            nc.scalar.copy(srow[3][:], pTk[CIN:CIN + 1, 0:128])
            pstq = props.tile([1, 512], F32, name="pstq", tag="st", bufs=2)
            nc.tensor.matmul(pstq[:, 0:128], ones64[:], wT_q[:], start=True, stop=True)
            nc.scalar.copy(srow[0][:], pstq[:, 0:128])
            pstk = props.tile([1, 512], F32, name="pstk", tag="st", bufs=2)
            nc.tensor.matmul(pstk[:, 0:128], ones64[:], wT_k[:], start=True, stop=True)
            nc.scalar.copy(srow[1][:], pstk[:, 0:128])
            for i4 in range(4):
                pcl = props.tile([128, 512], F32, name=f"pcl{i4}", tag="st", bufs=2)
                nc.tensor.transpose(pcl[:, 0:1], srow[i4][:], ident[0:1, 0:1])
                nc.scalar.copy(stc[:, i4:i4 + 1], pcl[:, 0:1])
            # ---- stats math -> g (scale) and bias rows for q, k ----
            mu = prow.tile([128, 2], F32)    # mu_q, mu_k
            nc.vector.tensor_scalar_mul(mu[:], stc[:, 2:4], 1.0 / S)
            e2 = prow.tile([128, 2], F32)
            nc.vector.tensor_scalar_mul(e2[:], stc[:, 0:2], 1.0 / S)
            musq = prow.tile([128, 2], F32)
            nc.vector.tensor_tensor(out=musq[:], in0=mu[:], in1=mu[:], op=ALU.mult)
            vareps = prow.tile([128, 2], F32)
            nc.vector.tensor_tensor(out=vareps[:], in0=e2[:], in1=musq[:],
                                    op=ALU.subtract)
            nc.vector.tensor_scalar_add(vareps[:], vareps[:], EPS)
            # rsqrt via sqrt + iterative-divide reciprocal (exact-ish)
            sq_t = prow.tile([128, 2], F32)
            nc.scalar.activation(sq_t[:], vareps[:], AF.Sqrt)
            g = prow.tile([128, 2], F32)
            nc.vector.reciprocal(g[:], sq_t[:])
            nc.vector.tensor_scalar_mul(g[:, 0:1], g[:, 0:1], SCALE)  # fold *16
            bias = prow.tile([128, 2], F32)
            nc.vector.tensor_tensor(out=bias[:], in0=mu[:], in1=g[:], op=ALU.mult)
            nc.vector.tensor_scalar_mul(bias[:], bias[:], -1.0)

            # ---- scaled weights + bias row -> stationary [65, 128] f32r ----
            swqT = prow.tile([128, CIN], F32)
            swkT = prow.tile([128, CIN], F32)
            nc.vector.tensor_scalar_mul(swqT[:], wqT[:], g[:, 0:1])
            nc.vector.tensor_scalar_mul(swkT[:], wkT[:], g[:, 1:2])
            Wq_st = prow.tile([CIN + 1, 128], F32R)
            Wk_st = prow.tile([CIN + 1, 128], F32R)
            pSWq = props.tile([CIN, 512], F32, name="pSWq", tag="st", bufs=2)
            nc.tensor.transpose(pSWq[:, 0:128], swqT[:], ident[:])
            nc.scalar.copy(Wq_st[0:CIN, :], pSWq[:, 0:128])
            pSWk = props.tile([CIN, 512], F32, name="pSWk", tag="st", bufs=2)
            nc.tensor.transpose(pSWk[:, 0:128], swkT[:], ident[:])
            nc.scalar.copy(Wk_st[0:CIN, :], pSWk[:, 0:128])
            pBq = props.tile([1, 512], F32, name="pBq", tag="st", bufs=2)
            nc.tensor.transpose(pBq[:, 0:128], bias[:, 0:1], ident[:])
            nc.scalar.copy(Wq_st[CIN:CIN + 1, :], pBq[:, 0:128])
            pBk = props.tile([1, 512], F32, name="pBk", tag="st", bufs=2)
            nc.tensor.transpose(pBk[:, 0:128], bias[:, 1:2], ident[:])
            nc.scalar.copy(Wk_st[CIN:CIN + 1, :], pBk[:, 0:128])
            wv_r = prow.tile([CIN, DH], F32R)
            nc.vector.tensor_copy(wv_r[:], wv[:])

            # ---- projections: qhat16 / khat / v  (f32r, 1 cyc/row),
            #      [*,512] tiles, one tag per tensor so q/k/v pipeline ----
            vT = per.tile([DH, S], F32) if STAGE == 2 else prow.tile([DH, S], F32)
            vTkeep = vT
            for et in range(8):
                sl = bass.ts(et, 512)
                pq = props.tile([128, 512], F32, name=f"pq{et}", tag="pjq", bufs=2)
                nc.tensor.matmul(pq[:], Wq_st[:], xTr[:, sl], start=True, stop=True)
                nc.scalar.copy(qF[:, sl], pq[:])             # f16 rounds
                pk = props.tile([128, 512], F32, name=f"pk{et}", tag="pjk", bufs=2)
                nc.tensor.matmul(pk[:], Wk_st[:], xTr[:, sl], start=True, stop=True)
                nc.vector.tensor_copy(kF[:, sl], pk[:])
                pv = props.tile([DH, 512], F32, name=f"pv{et}", tag="pjv", bufs=2)
                nc.tensor.matmul(pv[:], wv_r[:], xTr[0:CIN, sl], start=True, stop=True)
                nc.scalar.copy(vT[:, sl], pv[:])
            if STAGE == 1:
                dbg = prow.tile([CIN, S], F32)
                for qt2 in range(4):
                    qs2 = bass.ts(qt2, 1024)
                    nc.scalar.copy(dbg[0:32, qs2], qF[0:32, qs2])
                    nc.scalar.copy(dbg[32:64, qs2], kF[0:32, qs2])
                nc.sync.dma_start(out_d[:], dbg[:])
            # kP: rows 0-31 & 64-95 = khat f16, rows 32/96 = -1
            nc.sync.dma_start(kP[0:32, :], kF[0:32, :])
            nc.sync.dma_start(kP[64:96, :], kF[0:32, :])
            neg1 = prow.tile([1, S], F16)
            nc.vector.memset(neg1[:], -1.0)
            nc.sync.dma_start(kP[32:33, :], neg1[:])
            nc.sync.dma_start(kP[96:97, :], neg1[:])
            # qPc data rows for ALL chunks (m-rows arrive per chunk)
            for c in range(NCH):
                cs = bass.ts(c, CHUNK)
                nc.sync.dma_start(qPc[c][0:32, :], qF[0:32, cs])
                nc.sync.dma_start(qPc[c][64:96, :], qF[0:32, cs])

            # ---- vaug: per-j-block transposed v | ones ----
            for jb in range(NB):
                pvt = props.tile([128, 512], F32, name=f"pvt{jb}", tag="pjq",
                                 bufs=2)
                nc.tensor.transpose(pvt[:, 0:DH], vT[:, bass.ts(jb, 128)],
                                    ident[0:DH, 0:DH])
                nc.vector.tensor_copy(vaug[:, bass.ds(33 * jb, DH)],
                                      pvt[:, 0:DH])
        if STAGE == 2:
            with tc.tile_pool(name="dbg2", bufs=1) as dbgp:
                dbg = dbgp.tile([CIN, S], F32)
                nc.vector.memset(dbg[:], 0.0)
                nc.vector.tensor_copy(dbg[0:DH, :], vTkeep[:])
                nc.vector.tensor_copy(dbg[DH:DH + 33, 0:33 * NB], vaug[0:33, :])
                nc.sync.dma_start(out_d[:], dbg[:])
        # ================= main loop =================
        with tc.tile_pool(name="mcolp", bufs=2) as mcol_pool, \
             tc.tile_pool(name="ptp", bufs=8) as pt_pool, \
             tc.tile_pool(name="ppn", bufs=2) as pn_pool:

            mparts_tiles = {}
            psA_pool_ref = [None]

            def emit_passA_quarter(ib, quarter, use_lse):
                """Pass A for i-block ib, j in [1024q, 1024q+1024): one
                [128,1024] psum quarter, 2-way row-strip pair (alternating
                strip sets per quarter); DVE reduce or ACT lse scan."""
                if quarter == 0:
                    mparts_tiles[ib] = mcol_pool.tile([128, 4], F32,
                                                      name=f"mp{ib}", tag="mparts",
                                                      bufs=3)
                mparts = mparts_tiles[ib]
                psA = psA_pool_ref[0].tile([128, 1024], F32,
                                           name=f"psA{ib}_{quarter}", tag="psA")
                boff = 64 * (quarter % 2)
                for r in range(2):
                    nc.tensor.matmul(
                        psA[:, bass.ts(r, 512)],
                        qF[bass.ds(boff + 32 * r, 32), bass.ts(ib, 128)],
                        kF[bass.ds(boff + 32 * r, 32),
                           bass.ds(1024 * quarter + 512 * r, 512)],
                        start=True, stop=True,
                        tile_position=(boff + 32 * r, 0),
                    )
                if use_lse:
                    ju = pt_pool.tile([128, 1024], BF16, name=f"jl{ib}_{quarter}",
                                      tag="julse", bufs=2)
                    nc.scalar.activation(ju[:], psA[:], AF.Exp, scale=0.0625,
                                         bias=lse_bias[:],
                                         accum_out=mparts[:, quarter:quarter + 1])
                else:
                    nc.vector.reduce_max(mparts[:, quarter:quarter + 1], psA[:],
                                         axis=mybir.AxisListType.X)
                if quarter < 3:
                    return
                mparts_tiles.pop(ib)
                mcol = m8[(ib // 8)][:, ib % 8:ib % 8 + 1]
                if use_lse:
                    l8 = mcol_pool.tile([128, 1], F32, name=f"l8{ib}", tag="l8")
                    nc.vector.reduce_sum(l8[:], mparts[:], axis=mybir.AxisListType.X)
                    lnl = mcol_pool.tile([128, 1], F32, name=f"lnl{ib}", tag="lnl8")
                    nc.scalar.activation(lnl[:], l8[:], AF.Ln)
                    # mhat = 16*ln(sum exp((s-500)/16)) + 500 - 30
                    nc.vector.tensor_scalar(out=mcol, in0=lnl[:], scalar1=16.0,
                                            scalar2=470.0, op0=ALU.mult, op1=ALU.add)
                else:
                    nc.vector.reduce_max(mcol, mparts[:], axis=mybir.AxisListType.X)

            def stage_mrow(c):
                """Batch-transpose m8[c] -> f16 rows 32/96 of qPc[c]."""
                pm = psA_pool_ref[0].tile([8, 128], F32, name=f"pm{c}", tag="psA")
                nc.tensor.transpose(pm[:], m8[c][:], ident[:])
                mrow = mcol_pool.tile([8, 128], F16, name=f"mrow{c}", tag="mrow")
                nc.scalar.copy(mrow[:], pm[:])
                for i8 in range(8):
                    nc.gpsimd.dma_start(qPc[c][32:33, bass.ts(i8, 128)],
                                        mrow[i8:i8 + 1, :])
                    nc.gpsimd.dma_start(qPc[c][96:97, bass.ts(i8, 128)],
                                        mrow[i8:i8 + 1, :])

            def lse_ib(ib):
                return (ib % 8) >= 8 - (NLSE + 3) // 4 if NLSE else False

            # ---- peel: pass A chunk 0 in its own psum scope (deep bufs;
            #      3 of 8 i-blocks on ACT-lse to split the scan) ----
            with tc.tile_pool(name="psApeel", bufs=4, space="PSUM") as peel_pool:
                psA_pool_ref[0] = peel_pool
                for ib in (0, 4, 1, 5, 2, 6, 3, 7):
                    for q in range(4):
                        emit_passA_quarter(ib, q, ib >= 4)
                stage_mrow(0)

            if STAGE == 3:
                with tc.tile_pool(name="psA3", bufs=2, space="PSUM") as psA_pool:
                    psA_pool_ref[0] = psA_pool
                    for ch2 in range(1, NCH):
                        for jb in range(NB):
                            ib = 8 * ch2 + jb // 4
                            emit_passA_quarter(ib, jb % 4, lse_ib(ib))
                        stage_mrow(ch2)
                    dbg = pn_pool.tile([128, 32], F32, name="dbgm", tag="dbgm")
                    for c2 in range(NCH):
                        nc.vector.tensor_copy(dbg[:, bass.ts(c2, 8)], m8[c2][:])
                    nc.vector.memset(yT[:], 0.0)
                    nc.vector.tensor_copy(yT[0:64, 0:32], dbg[0:64, :])
                    nc.vector.tensor_copy(yT[:, 32:64], dbg[64:128, :])
                    for c2 in range(NCH):
                        nc.sync.dma_start(out_d[:, bass.ts(c2, CHUNK)], yT[:])

            mainloop_on = STAGE >= 4
            if mainloop_on:
              with tc.tile_pool(name="psB", bufs=1, space="PSUM") as psB_pool, \
                   tc.tile_pool(name="psA", bufs=2, space="PSUM") as psA_pool, \
                   tc.tile_pool(name="psAV", bufs=1, space="PSUM") as psAV_pool, \
                   tc.tile_pool(name="psO", bufs=1, space="PSUM") as psO_pool:
                psA_pool_ref[0] = psA_pool

                for ch in range(NCH):
                    avh = psAV_pool.tile([128, 512], F32, name=f"avh{ch}", tag="av")
                    pts = {}
                    for jb in range(NB):
                        # pass A of next chunk: quarters shifted ~4 early so
                        # the mhat staging overlaps jb 28..31
                        if ch + 1 < NCH:
                            qs = [jb] if jb < 28 else []
                            if jb < 4:
                                qs.append(28 + jb)
                            for qq in qs:
                                ib = 8 * (ch + 1) + qq // 4
                                emit_passA_quarter(ib, qq % 4, lse_ib(ib))
                            if jb == 28:
                                stage_mrow(ch + 1)
                        lastch = (ch == NCH - 1)
                        use_a = lastch and (jb % 3) < 2
                        psB = (psA_pool if use_a else psB_pool).tile(
                            [128, CHUNK], F32, name=f"psB{ch}_{jb}",
                            tag="psA" if use_a else "psB")
                        nc.tensor.matmul(psB[:, 0:512], kP[0:33, bass.ts(jb, 128)],
                                         qPc[ch][0:33, 0:512], start=True, stop=True,
                                         tile_position=(0, 0))
                        nc.tensor.matmul(psB[:, 512:1024],
                                         kP[64:97, bass.ts(jb, 128)],
                                         qPc[ch][64:97, 512:1024], start=True,
                                         stop=True, tile_position=(64, 0))
                        pt = pt_pool.tile([128, CHUNK], BF16, name=f"pt{ch}_{jb}",
                                          tag="pt")
                        nc.scalar.activation(pt[:], psB[:], AF.Exp)
                        pts[jb] = pt
                        # AV software-pipelined three jb behind the exp
                        if jb >= 3:
                            jp = jb - 3
                            ptp = pts.pop(jp)
                            nc.tensor.matmul(avh[0:33, :],
                                             vaug[:, bass.ds(33 * jp, 33)],
                                             ptp[:, 0:512], start=(jp == 0),
                                             stop=False, tile_position=(0, 0))
                            nc.tensor.matmul(avh[64:97, :],
                                             vaug[:, bass.ds(33 * jp, 33)],
                                             ptp[:, 512:1024], start=(jp == 0),
                                             stop=False, tile_position=(0, 64))
                    for jp in (NB - 3, NB - 2, NB - 1):
                        ptp = pts.pop(jp)
                        nc.tensor.matmul(avh[0:33, :], vaug[:, bass.ds(33 * jp, 33)],
                                         ptp[:, 0:512], start=False,
                                         stop=(jp == NB - 1), tile_position=(0, 0))
                        nc.tensor.matmul(avh[64:97, :],
                                         vaug[:, bass.ds(33 * jp, 33)],
                                         ptp[:, 512:1024], start=False,
                                         stop=(jp == NB - 1),
                                         tile_position=(0, 64))

                    # ---- out-stage for this chunk (own psum bank) ----
                    lrow = pn_pool.tile([1, CHUNK], F32, name=f"lrow{ch}",
                                        tag="lrow")
                    nc.vector.tensor_copy(lrow[:, 0:512], avh[32:33, :])
                    nc.vector.tensor_copy(lrow[:, 512:1024], avh[96:97, :])
                    lnl = pn_pool.tile([1, CHUNK], F32, name=f"lnl{ch}", tag="lnl")
                    nc.scalar.activation(lnl[:], lrow[:], AF.Ln)
                    linv = pn_pool.tile([1, CHUNK], F32R, name=f"linv{ch}",
                                        tag="linv")
                    nc.scalar.activation(linv[:], lnl[:], AF.Exp, scale=-1.0)
                    for half in range(2):
                        hs = bass.ts(half, 512)
                        pl = psO_pool.tile([33, 512], F32, name=f"pl{ch}_{half}",
                                           tag="psO")
                        nc.tensor.matmul(pl[:], ones33[:], linv[:, hs],
                                         start=True, stop=True)
                        linv_rep = pn_pool.tile([33, 512], F32,
                                                name=f"lr{ch}_{half}", tag="linvrep")
                        nc.scalar.copy(linv_rep[:], pl[:])
                        pn = pn_pool.tile([33, 512], F32R, name=f"pn{ch}_{half}",
                                          tag="pn")
                        nc.vector.tensor_tensor(out=pn[:],
                                                in0=avh[bass.ds(64 * half, 33), :],
                                                in1=linv_rep[:], op=ALU.mult)
                        psY = psO_pool.tile([CIN, 512], F32, name=f"psY{ch}_{half}",
                                            tag="psO")
                        nc.tensor.matmul(psY[:], wo_r[:], pn[:], start=True,
                                         stop=True)
                        nc.scalar.copy(yT[:, hs], psY[:])
                        nc.sync.dma_start(
                            out_d[:, bass.ds(CHUNK * ch + 512 * half, 512)],
                            yT[:, hs])

    nc.compile()
    return nc


def _get_compiled():
    global _compiled
    if _compiled is None:
        _compiled = _build()
    return _compiled


def kernel(input, w_qkv, w_out, b_out):
    input = np.asarray(input, dtype=np.float32)
    w_qkv = np.asarray(w_qkv, dtype=np.float32)
    w_out = np.asarray(w_out, dtype=np.float32)
    b_out = np.asarray(b_out, dtype=np.float32)
    b, x, y, z, c = input.shape
    assert (b, x, y, z, c) == (2, 16, 16, 16, 64)
    hid = HEADS * DH

    in_maps = []
    for core in range(8):
        bb, h = divmod(core, HEADS)
        xf = input[bb].reshape(S, CIN)                    # [4096, 64]
        xTa = np.vstack([xf.T, np.ones((1, S), np.float32)])
        xb = xf.reshape(32, 128, CIN)
        xNb = np.concatenate([xb, np.ones((32, 128, 1), np.float32)], axis=2)
        xNb = np.ascontiguousarray(xNb.transpose(1, 0, 2).reshape(128, 32 * 65))
        wq = np.tile(w_qkv[:, h * DH:(h + 1) * DH], (1, 4))
        wk = np.tile(w_qkv[:, hid + h * DH: hid + (h + 1) * DH], (1, 4))
        wv = np.ascontiguousarray(w_qkv[:, 2 * hid + h * DH: 2 * hid + (h + 1) * DH])
        wo = np.vstack([w_out[h * DH:(h + 1) * DH, :], b_out[None, :] / HEADS])
        in_maps.append({
            "xTa": np.ascontiguousarray(xTa),
            "xNb": xNb,
            "wqT": np.ascontiguousarray(wq.T),
            "wkT": np.ascontiguousarray(wk.T),
            "wv": wv,
            "wo": np.ascontiguousarray(wo),
        })

    global _last_in_maps
    _last_in_maps = in_maps
    nc = _get_compiled()
    res = run_bass_kernel_spmd(nc, in_maps, core_ids=list(range(8)))
    out = np.zeros((b, S, CIN), dtype=np.float32)
    for core in range(8):
        bb = core // HEADS
        out[bb] += res.results[core]["out"].T
    return out.reshape(b, x, y, z, CIN)


if __name__ == "__main__":
    rng = np.random.default_rng(0)
    inp = rng.standard_normal((2, 16, 16, 16, 64), dtype=np.float32)
    wqkv = rng.standard_normal((64, 384), dtype=np.float32) / 8.0
    wout = rng.standard_normal((128, 64), dtype=np.float32) / np.sqrt(128)
    bout = np.zeros(64, dtype=np.float32)
    o = kernel(inp, wqkv, wout, bout)
    print("kernel output shape:", o.shape)
